# revision 31
# baseline (speedup 1.0000x reference)
"""AELoss (associative embedding loss) distributed Bass kernel for TRN2.

Problem: ebd_batch [16, 544, 128, 128] f32, kpts [16, 20, 17, 2] f32.
  vecs[b,p,k,:] = ebd[b, k*32:(k+1)*32, y(b,p,k), x(b,p,k)]  (y=floor(ky*128))
  means = vecs.mean(parts); pull/push L1 stats -> scalar loss.

Strategy: pure data parallel over batch (2 batches/core on 8 cores). The
essential data is only 5440 32-float vectors out of 570MB, so instead of
streaming, each core dma_gathers 256B chunks (the minimum indexable unit)
around each needed element — 20 calls spread over the 4 SWDGE queues (each
queue drains at roughly one SDMA engine's line rate) — then extracts the
exact element on-chip with a one-hot compare+reduce overlapped per call.
Indices are computed on device from kpts (exact floor via int bit
manipulation). Per-core partial losses are summed on host.

Layout cheat sheet (per core):
  B_L=2 local batches, P=20 people, parts padded 17->20 = 5 groups (g) of 4
  (kl), D=32. call q = b*5+g (2 gather halves each). c_local = kl*32+d.
  E[c_local, qp] with qp = q*20+p holds vec(b, p, k=4g+kl, d).
  gather idx (int16) = c_eff*256 + 2*y + (x>=64), window = 128 planes (8MB).
"""

import sys

sys.path.insert(0, "/opt/trn_rl_repo")

import numpy as np
import ml_dtypes

import concourse.mybir as mybir
from concourse.ap import AP
from concourse.bacc import Bacc

F32 = mybir.dt.float32
BF16 = mybir.dt.bfloat16
I32 = mybir.dt.int32
I16 = mybir.dt.int16

B, CH, H, W = 16, 544, 128, 128
D = 32
N_PARTS = 17
P = 20
N_CORES = 8
B_L = B // N_CORES          # 2 local batches
NG = 5                      # part groups of 4 (parts padded to 20)
NQ = B_L * NG               # 10 logical gather calls (x2 halves)
NI = P * 128                # 2560 idxs per logical call
NC = NQ * P                 # 200 E columns
PLANE = H * W
ELEM = 64                   # gathered chunk = 64 f32 = 256B
CHUNKS_PER_PLANE = PLANE // ELEM  # 256

# loss = mean_b (push_b + pull_b)/2 ; per-core out = sum_local_b (...)/2,
# host divides by B. pull_b = sum_all|d|/544 ; push_b = sum|md|/12800.
C_PULL = 1.0 / (544.0 * 2.0)
C_PUSH = 1.0 / (12800.0 * 2.0)


def _host_consts():
    """Constant tensors DMA'd to every core."""
    # A term of the gather index: A[Pr, J] = c_eff * 256 where
    # i = (J%160)*16 + Pr%16, c_local = i % 128, call q = J // 160,
    # c_eff = c_local%32 if q%5==4 else c_local  (last group: only part 16
    # is real; pad kls duplicate part16's chunk so addresses stay in range).
    Pr = np.arange(128)[:, None]
    J = np.arange(NQ * 160)[None, :]
    i = (J % 160) * 16 + (Pr % 16)
    c_local = i % 128
    q = J // 160
    c_eff = np.where(q % NG == NG - 1, c_local % 32, c_local)
    A = (c_eff * 256).astype(np.float32)

    IOTA = np.tile(np.arange(ELEM, dtype=np.float32)[None, :], (128, 1))

    # E4T[kl, c] = 1 if c//32 == kl   (broadcasts xm to 128 partitions)
    E4T = (np.arange(128)[None, :] // 32 == np.arange(4)[:, None]).astype(
        ml_dtypes.bfloat16
    )

    # SEL [128, 64]: cols 0:32 full-group mean weights, cols 32:64 last group
    c = np.arange(128)[:, None]
    d = np.arange(D)[None, :]
    sel_full = (c % 32 == d).astype(np.float32) / N_PARTS
    sel_last = sel_full * (c < 32)
    SEL = np.concatenate([sel_full, sel_last], axis=1).astype(np.float32)

    # RT[d, c] = 1 if c%32 == d  (broadcast means over part groups)
    RT = (np.arange(128)[None, :] % 32 == np.arange(D)[:, None]).astype(
        np.float32
    )

    ONES_COL = np.ones((128, 1), dtype=np.float32)   # lhsT for partition sum
    # SEL4[j, kl*128 + c] = 1 if j == kl : row-broadcast selector
    SEL4 = np.zeros((4, 4 * 128), dtype=ml_dtypes.bfloat16)
    for kl in range(4):
        SEL4[kl, kl * 128:(kl + 1) * 128] = 1.0
    W6 = np.array([[C_PULL, C_PULL, C_PULL, C_PULL, C_PUSH, C_PUSH]],
                  dtype=np.float32)
    return dict(A=A, IOTA=IOTA, E4T=E4T, SEL=SEL, RT=RT,
                ONES_COL=ONES_COL, SEL4=SEL4, W6=W6)


def _kpts_prep(kpts_shard):
    """[B_L, P, 17, 2] -> [4, 400] f32: V[kl, c*200 + (b*5+g)*20 + p] =
    kpts[b, p, min(4g+kl, 16), c]."""
    k_ids = np.minimum(np.arange(P), N_PARTS - 1)  # 0..16,16,16,16
    kp = kpts_shard[:, :, k_ids, :]                # [B_L, P, 20, 2]
    kp = kp.reshape(B_L, P, NG, 4, 2).transpose(3, 4, 0, 2, 1)
    return np.ascontiguousarray(kp.reshape(4, 2 * B_L * NG * P)).astype(
        np.float32
    )


def build_graph():
    nc = Bacc(num_swdge_queues=4, dynamic_dma_scratch_size=65536)

    ebd = nc.declare_dram_parameter("ebd", [B_L, CH, H, W], F32, isOutput=False)
    kp = nc.declare_dram_parameter("kp", [4, 400], F32, isOutput=False)
    A_d = nc.declare_dram_parameter("A", [128, NQ * 160], F32, isOutput=False)
    IO_d = nc.declare_dram_parameter("IOTA", [128, ELEM], F32, isOutput=False)
    E4_d = nc.declare_dram_parameter("E4T", [4, 128], BF16, isOutput=False)
    SEL_d = nc.declare_dram_parameter("SEL", [128, 2 * D], F32, isOutput=False)
    RT_d = nc.declare_dram_parameter("RT", [D, 128], F32, isOutput=False)
    OC_d = nc.declare_dram_parameter("ONES_COL", [128, 1], F32, isOutput=False)
    S4_d = nc.declare_dram_parameter("SEL4", [4, 4 * 128], BF16, isOutput=False)
    W6_d = nc.declare_dram_parameter("W6", [1, 6], F32, isOutput=False)
    out_ext = nc.declare_dram_parameter("out", [1], F32, isOutput=True)

    from contextlib import ExitStack

    ctx = ExitStack()
    with ctx:
        sb = lambda name, shape, dt=F32: ctx.enter_context(
            nc.sbuf_tensor(name, shape, dt)
        )
        ps = lambda name, shape: ctx.enter_context(
            nc.psum_tensor(name, shape, F32)
        )

        Vt = sb("Vt", [4, 400])            # kpts coords (kl x (c,b,g,p))
        U32 = sb("U32", [4, 400], I32)     # scratch int views
        SH = sb("SH", [4, 400], I32)
        YI = sb("YI", [4, 400], I32)       # floor(v*128) as int32
        Yf = sb("Yf", [4, 400])            # floor(v*128) as f32
        Gg = sb("Gg", [4, 200])            # (x >= 64) as 0/1 f32
        Gm = sb("Gm", [4, 200])            # -64*g
        BF1 = sb("BF1", [4, 200])          # 2y
        BASEf = sb("BASEf", [4, 200], BF16)  # 2y + g  (<= 255, bf16-exact)
        XMf = sb("XMf", [4, 200], BF16)    # x % 64   (<= 63, bf16-exact)
        At = sb("At", [128, NQ * 160])     # A const f32
        IDX = sb("IDX", [128, NQ * 160], I16)
        It = sb("It", [128, ELEM])
        E4t = sb("E4t", [4, 128], BF16)
        SELt = sb("SELt", [128, 2 * D])
        RTt = sb("RTt", [D, 128])
        OCt = sb("OCt", [128, 1])
        S4t = sb("S4t", [4, 4 * 128], BF16)
        W6t = sb("W6t", [1, 6])
        G = sb("G", [128, NQ * P * ELEM])  # gathered chunks
        XbS = sb("XbS", [128, NC])         # xm broadcast to c_local rows
        M1 = sb("M1", [128, (P // 2) * ELEM])  # one-hot scratch (per half)
        P1 = sb("P1", [128, (P // 2) * ELEM])  # product scratch
        E = sb("E", [128, NC])             # extracted vec values
        Mrep = sb("Mrep", [D, NC])         # means replicated over g
        DF = sb("DF", [128, NC])           # E - mean
        T = sb("T", [128, 6])              # cols 0:2 r1, 2:4 r2, 4:6 push
        FW = sb("FW", [1, 6])
        OUTs = sb("OUTs", [1, 1])

        Bps = ps("Bps", [128, 4 * 512])    # base broadcast (bank-padded)
        Xb = ps("Xb", [128, NC])
        Mps = ps("Mps", [D, 2 * P])
        MB = ps("MB", [128, NC])
        F = ps("F", [1, 6])

        sk = ctx.enter_context(nc.semaphore("sk"))   # kpts DMA
        sa = ctx.enter_context(nc.semaphore("sa"))   # A const DMA
        ss = ctx.enter_context(nc.semaphore("ss"))   # small const DMAs
        gds = [[ctx.enter_context(nc.semaphore(f"gd{q}_{h}"))
                for h in range(2)] for q in range(NQ)]
        sv = ctx.enter_context(nc.semaphore("sv"))   # vector
        sp = ctx.enter_context(nc.semaphore("sp"))   # PE
        block = ctx.enter_context(nc.Block())

        MS = {}

        @block.vector
        def _(vec):
            AL = mybir.AluOpType
            cnt = [0]

            def fin(inst):
                inst.then_inc(sv, 1)
                cnt[0] += 1

            def w():
                # same-engine RAW/WAR guard: DVE has no pipeline interlocks
                vec.wait_ge(sv, cnt[0])

            fin(nc.vector.memset(T[:], 0.0))
            vec.wait_ge(sk, 16)  # kpts loaded

            # exact floor(v*128) = mant >> min(143 - exp, 31); run the
            # y-half first so BASEf (which gates PE -> idx -> gathers)
            # is ready ~2.5us earlier, then the x-half overlaps PE.
            def floor_half(lo, hi):
                uh = Vt[:, lo:hi].bitcast(I32)
                fin(nc.vector.tensor_scalar(
                    out=U32[:, lo:hi], in0=uh, scalar1=23, scalar2=None,
                    op0=AL.logical_shift_right,
                ))
                w()
                fin(nc.vector.tensor_scalar(
                    out=SH[:, lo:hi], in0=U32[:, lo:hi], scalar1=-1,
                    scalar2=143, op0=AL.mult, op1=AL.add,
                ))
                w()
                fin(nc.vector.tensor_scalar(
                    out=SH[:, lo:hi], in0=SH[:, lo:hi], scalar1=31,
                    scalar2=None, op0=AL.min,
                ))
                w()
                fin(nc.vector.tensor_scalar(
                    out=U32[:, lo:hi], in0=uh, scalar1=0x7FFFFF,
                    scalar2=0x800000, op0=AL.bitwise_and, op1=AL.bitwise_or,
                ))
                w()
                fin(nc.vector.tensor_tensor(
                    out=YI[:, lo:hi], in0=U32[:, lo:hi], in1=SH[:, lo:hi],
                    op=AL.logical_shift_right,
                ))
                w()
                fin(nc.vector.tensor_copy(out=Yf[:, lo:hi],
                                          in_=YI[:, lo:hi]))

            floor_half(0, 200)       # y coords
            fin(nc.vector.tensor_scalar(
                out=Gg[:], in0=Vt[:, 200:400], scalar1=0.5, scalar2=None,
                op0=AL.is_ge,
            ))
            w()
            fin(nc.vector.tensor_scalar(
                out=BF1[:], in0=Yf[:, 0:200], scalar1=2.0, scalar2=None,
                op0=AL.mult,
            ))
            w()
            fin(nc.vector.tensor_tensor(
                out=BASEf[:], in0=BF1[:], in1=Gg[:], op=AL.add
            ))
            MS["base"] = cnt[0]
            floor_half(200, 400)     # x coords (overlaps PE Bps matmuls)
            fin(nc.vector.tensor_scalar(
                out=Gm[:], in0=Gg[:], scalar1=-64.0, scalar2=None, op0=AL.mult
            ))
            w()
            fin(nc.vector.tensor_tensor(
                out=XMf[:], in0=Yf[:, 200:400], in1=Gm[:], op=AL.add
            ))
            MS["xm"] = cnt[0]
            # idx = A + Bps  (even/odd cols; in1 reads PSUM with 3D AP),
            # emitted in two q-halves so the first gathers can issue early
            vec.wait_ge(sa, 16)   # At
            vec.wait_ge(sp, 4)    # Bps
            bp0 = Bps[:]
            HQ = NQ // 2
            for qh in range(2):
                for par in range(2):
                    # out/in0 enumerate (q, p, kl) at parity `par`;
                    # in1 reads Bps[c, kl*512 + q*20 + p] in the same order.
                    in1 = AP(bp0.tensor, bp0.offset + qh * HQ * P,
                             [bp0.ap[0], [20, HQ], [1, P], [512, 4]])
                    out = AP(IDX[:].tensor,
                             IDX[:].offset + par + qh * HQ * 160,
                             [IDX[:].ap[0], [160, HQ], [8, P], [2, 4]])
                    in0 = AP(At[:].tensor,
                             At[:].offset + par + qh * HQ * 160,
                             [At[:].ap[0], [160, HQ], [8, P], [2, 4]])
                    fin(nc.vector.tensor_tensor(
                        out=out, in0=in0, in1=in1, op=AL.add
                    ))
                if qh == 0:
                    MS["idx0"] = cnt[0]
            MS["idxdone"] = cnt[0]
            vec.wait_ge(sp, 5)
            fin(nc.vector.tensor_copy(out=XbS[:], in_=Xb[:]))
            MS["eslice"] = []
            # per-half extraction: one-hot compare + multiply + reduce
            HP = P // 2
            for q in range(NQ):
                for h in range(2):
                    vec.wait_ge(gds[q][h], 16)
                    if q == 0 and h == 0:
                        vec.wait_ge(ss, 112)  # IOTA et al
                    it = It[:]
                    iota_b = AP(it.tensor, it.offset,
                                [it.ap[0], [0, HP], [1, ELEM]])
                    xb = XbS[:, q * P + h * HP:q * P + (h + 1) * HP]
                    xb_b = AP(xb.tensor, xb.offset,
                              [xb.ap[0], [1, HP], [0, ELEM]])
                    w()
                    fin(nc.vector.tensor_tensor(
                        out=M1[:].rearrange("p (a b) -> p a b", b=ELEM),
                        in0=iota_b, in1=xb_b, op=AL.is_equal,
                    ))
                    w()
                    goff = (q * P + h * HP) * ELEM
                    fin(nc.vector.tensor_tensor(
                        out=P1[:], in0=G[:, goff:goff + HP * ELEM],
                        in1=M1[:], op=AL.mult,
                    ))
                    w()
                    fin(nc.vector.tensor_reduce(
                        out=E[:, q * P + h * HP:q * P + (h + 1) * HP],
                        in_=P1[:].rearrange("p (a b) -> p a b", b=ELEM),
                        axis=mybir.AxisListType.X, op=AL.add,
                    ))
                MS["eslice"].append(cnt[0])
            # tail: means -> pull/push partial sums into T's columns
            vec.wait_ge(sp, 15)
            mp = Mps[:]
            fin(nc.vector.tensor_copy(
                out=Mrep[:],
                in_=AP(mp.tensor, mp.offset,
                       [mp.ap[0], [P, 2], [0, NG], [1, P]]),
            ))
            MS["mrep"] = cnt[0]
            vec.wait_ge(sp, 16)
            fin(nc.vector.tensor_tensor(
                out=DF[:], in0=E[:], in1=MB[:], op=AL.subtract
            ))
            df = DF[:]
            w()
            fin(nc.vector.tensor_reduce(
                out=T[:, 0:2],
                in_=AP(df.tensor, df.offset,
                       [df.ap[0], [5 * P, 2], [P, 4], [1, P]]),
                axis=mybir.AxisListType.XY, op=AL.add,
                apply_absolute_value=True,
            ))
            df32 = DF[0:32, :]
            w()
            fin(nc.vector.tensor_reduce(
                out=T[0:32, 2:4],
                in_=AP(df32.tensor, df32.offset + 4 * P,
                       [df32.ap[0], [5 * P, 2], [1, P]]),
                axis=mybir.AxisListType.X, op=AL.add,
                apply_absolute_value=True,
            ))
            # push: pairwise |m_p - m_q| — in0 from Mps (PSUM), in1 from the
            # SBUF replica Mrep (g=0 block holds means at cols b*100+p)
            mr = Mrep[:]
            in0 = AP(mp.tensor, mp.offset, [mp.ap[0], [P, 2], [1, P], [0, P]])
            in1 = AP(mr.tensor, mr.offset,
                     [mr.ap[0], [5 * P, 2], [0, P], [1, P]])
            # G is fully consumed by now — reuse it as pairwise-diff scratch
            pd_out = G[0:32, 0:2 * P * P].rearrange(
                "p (a b c) -> p a b c", a=2, b=P
            )
            fin(nc.vector.tensor_tensor(
                out=pd_out, in0=in0, in1=in1, op=AL.subtract
            ))
            w()
            fin(nc.vector.tensor_reduce(
                out=T[0:32, 4:6], in_=pd_out, axis=mybir.AxisListType.XY,
                op=AL.add, apply_absolute_value=True,
            ))
            MS["tdone"] = cnt[0]
            vec.wait_ge(sp, 17)
            fin(nc.vector.tensor_tensor(
                out=FW[:], in0=F[:], in1=W6t[:], op=AL.mult
            ))
            w()
            fin(nc.vector.tensor_reduce(
                out=OUTs[:], in_=FW[:], axis=mybir.AxisListType.X, op=AL.add
            ))
            MS["loss"] = cnt[0]

        @block.tensor
        def _(pe):
            # Bps: broadcast base rows to 128 partitions (4 bf16 matmuls)
            pe.wait_ge(sv, MS["base"])
            pe.wait_ge(ss, 112)
            for kl in range(4):
                nc.tensor.matmul(
                    out=Bps[:, kl * 512:kl * 512 + 200],
                    lhsT=S4t[:, kl * 128:(kl + 1) * 128],
                    rhs=BASEf[:],
                    start=True, stop=True,
                ).then_inc(sp, 1)
            # Xb: broadcast xm to c_local partitions
            pe.wait_ge(sv, MS["xm"])
            nc.tensor.matmul(
                out=Xb[:], lhsT=E4t[:], rhs=XMf[:], start=True, stop=True
            ).then_inc(sp, 1)
            # means: accumulate per-b over groups
            for q in range(NQ):
                b, g = divmod(q, NG)
                pe.wait_ge(sv, MS["eslice"][q])
                nc.tensor.matmul(
                    out=Mps[:, b * P:(b + 1) * P],
                    lhsT=SELt[:, D:2 * D] if g == NG - 1 else SELt[:, 0:D],
                    rhs=E[:, q * P:(q + 1) * P],
                    start=(g == 0), stop=(g == NG - 1),
                ).then_inc(sp, 1)
            # MB: broadcast means to [128, NC]
            pe.wait_ge(sv, MS["mrep"])
            nc.tensor.matmul(
                out=MB[:], lhsT=RTt[:], rhs=Mrep[:], start=True, stop=True
            ).then_inc(sp, 1)
            # F: partition sum of T
            pe.wait_ge(sv, MS["tdone"])
            nc.tensor.matmul(
                out=F[:], lhsT=OCt[:], rhs=T[:], start=True, stop=True
            ).then_inc(sp, 1)

        @block.gpsimd
        def _(gpsimd):
            gpsimd.wait_ge(sv, MS["idx0"])
            qq = 0
            for q in range(NQ):
                if q == NQ // 2:
                    gpsimd.wait_ge(sv, MS["idxdone"])
                b, g = divmod(q, NG)
                base = b * CH * PLANE + g * 128 * PLANE
                nrows = (CHUNKS_PER_PLANE * 128) if g < NG - 1 else (
                    CHUNKS_PER_PLANE * 32
                )
                in_ap = AP(ebd, base, [[ELEM, nrows], [1, ELEM]])
                # two halves (10 people each) spread over the 4 SWDGE queues
                for h in range(2):
                    half = NI // 2
                    off = q * P * ELEM + h * (P // 2) * ELEM
                    out_ap = G[:, off:off + (P // 2) * ELEM].rearrange(
                        "p (a b) -> p a b", b=ELEM
                    )
                    gpsimd.dma_gather(
                        out_ap=out_ap,
                        in_ap=in_ap,
                        idxs_ap=IDX[:, q * 160 + h * 80:
                                    q * 160 + (h + 1) * 80],
                        num_idxs=half,
                        num_idxs_reg=half,
                        elem_size=ELEM,
                        single_packet=False,
                        queue_num=qq % 4,
                    ).then_inc(gds[q][h], 16)
                    qq += 1

        @block.sync
        def _(sync):
            sync.dma_start(out=Vt[:], in_=kp[:]).then_inc(sk, 16)
            sync.dma_start(out=At[:], in_=A_d[:]).then_inc(sa, 16)
            sync.wait_ge(sv, MS["loss"])
            sync.dma_start(out=out_ext[:], in_=OUTs[0:1, 0:1]).then_inc(sk, 16)

        @block.scalar
        def _(scalar):
            for dst, src in (
                (It, IO_d), (E4t, E4_d), (SELt, SEL_d), (RTt, RT_d),
                (OCt, OC_d), (S4t, S4_d), (W6t, W6_d),
            ):
                scalar.dma_start(out=dst[:], in_=src[:]).then_inc(ss, 16)

    return nc


_CONSTS = None


def _run(ebd_batch: np.ndarray, kpts: np.ndarray, trace: bool = False):
    from concourse.bass_utils import run_bass_kernel_spmd

    global _CONSTS
    if _CONSTS is None:
        _CONSTS = _host_consts()
    consts = _CONSTS

    nc = build_graph()
    nc.finalize()

    in_maps = []
    for c in range(N_CORES):
        sl = slice(c * B_L, (c + 1) * B_L)
        m = dict(
            ebd=np.ascontiguousarray(ebd_batch[sl]).astype(np.float32),
            kp=_kpts_prep(kpts[sl].astype(np.float32)),
            A=consts["A"], IOTA=consts["IOTA"], E4T=consts["E4T"],
            SEL=consts["SEL"], RT=consts["RT"],
            ONES_COL=consts["ONES_COL"], SEL4=consts["SEL4"],
            W6=consts["W6"],
        )
        in_maps.append(m)

    res = run_bass_kernel_spmd(
        nc, in_maps, core_ids=list(range(N_CORES)), trace=trace
    )
    total = sum(float(res.results[c]["out"][0]) for c in range(N_CORES))
    return np.float32(total / B), res


def kernel(ebd_batch: np.ndarray, kpts: np.ndarray) -> np.ndarray:
    return _run(ebd_batch, kpts, trace=False)[0]


if __name__ == "__main__":
    np.random.seed(0)
    ebd = np.random.randn(B, CH, H, W).astype(np.float32)
    kk = np.random.rand(B, P, N_PARTS, 2).astype(np.float32)
    print(kernel(ebd, kk))


# revision 33
# speedup vs baseline: 1.2057x; 1.2057x over previous
"""AELoss (associative embedding loss) distributed Bass kernel for TRN2.

Problem: ebd_batch [16, 544, 128, 128] f32, kpts [16, 20, 17, 2] f32.
  vecs[b,p,k,:] = ebd[b, k*32:(k+1)*32, y(b,p,k), x(b,p,k)]  (y=floor(ky*128))
  means = vecs.mean(parts); pull/push L1 stats -> scalar loss.

Strategy: pure data parallel over batch (2 batches/core on 8 cores). The
essential data is only 5440 32-float vectors out of 570MB, so instead of
streaming, each core dma_gathers 256B chunks (the minimum indexable unit)
around each needed element — 20 calls spread over the 4 SWDGE queues (each
queue drains at roughly one SDMA engine's line rate) — then extracts the
exact element on-chip with a one-hot compare+reduce overlapped per call.
Indices are computed on device from kpts (exact floor via int bit
manipulation). Per-core partial losses are summed on host.

Layout cheat sheet (per core):
  B_L=2 local batches, P=20 people, parts padded 17->20 = 5 groups (g) of 4
  (kl), D=32. call q = b*5+g (2 gather halves each). c_local = kl*32+d.
  E[c_local, qp] with qp = q*20+p holds vec(b, p, k=4g+kl, d).
  gather idx (int16) = c_eff*256 + 2*y + (x>=64), window = 128 planes (8MB).
"""

import sys

sys.path.insert(0, "/opt/trn_rl_repo")

import numpy as np
import ml_dtypes

import concourse.mybir as mybir
from concourse.ap import AP
from concourse.bacc import Bacc

F32 = mybir.dt.float32
BF16 = mybir.dt.bfloat16
I32 = mybir.dt.int32
I16 = mybir.dt.int16

B, CH, H, W = 16, 544, 128, 128
D = 32
N_PARTS = 17
P = 20
N_CORES = 8
B_L = B // N_CORES          # 2 local batches
NG = 5                      # part groups of 4 (parts padded to 20)
NQ = B_L * NG               # 10 logical gather calls (x2 halves)
NI = P * 128                # 2560 idxs per logical call
NC = NQ * P                 # 200 E columns
PLANE = H * W
ELEM = 64                   # gathered chunk = 64 f32 = 256B
CHUNKS_PER_PLANE = PLANE // ELEM  # 256

# loss = mean_b (push_b + pull_b)/2 ; per-core out = sum_local_b (...)/2,
# host divides by B. pull_b = sum_all|d|/544 ; push_b = sum|md|/12800.
C_PULL = 1.0 / (544.0 * 2.0)
C_PUSH = 1.0 / (12800.0 * 2.0)


def _host_consts():
    """Constant tensors DMA'd to every core."""
    # A term of the gather index: A[Pr, J] = c_eff * 256 where
    # i = (J%160)*16 + Pr%16, c_local = i % 128, call q = J // 160,
    # c_eff = c_local%32 if q%5==4 else c_local  (last group: only part 16
    # is real; pad kls duplicate part16's chunk so addresses stay in range).
    Pr = np.arange(128)[:, None]
    J = np.arange(NQ * 160)[None, :]
    i = (J % 160) * 16 + (Pr % 16)
    c_local = i % 128
    q = J // 160
    c_eff = np.where(q % NG == NG - 1, c_local % 32, c_local)
    A = (c_eff * 256).astype(np.float32)

    IOTA = np.tile(np.arange(ELEM, dtype=np.float32)[None, :], (128, 1))

    # E4T[kl, c] = 1 if c//32 == kl   (broadcasts xm to 128 partitions)
    E4T = (np.arange(128)[None, :] // 32 == np.arange(4)[:, None]).astype(
        ml_dtypes.bfloat16
    )

    # SEL [128, 64]: cols 0:32 full-group mean weights, cols 32:64 last group
    c = np.arange(128)[:, None]
    d = np.arange(D)[None, :]
    sel_full = (c % 32 == d).astype(np.float32) / N_PARTS
    sel_last = sel_full * (c < 32)
    SEL = np.concatenate([sel_full, sel_last], axis=1).astype(np.float32)

    # RT[d, c] = 1 if c%32 == d  (broadcast means over part groups)
    RT = (np.arange(128)[None, :] % 32 == np.arange(D)[:, None]).astype(
        np.float32
    )

    ONES_COL = np.ones((128, 1), dtype=np.float32)   # lhsT for partition sum
    # SEL4[j, kl*128 + c] = 1 if j == kl : row-broadcast selector
    SEL4 = np.zeros((4, 4 * 128), dtype=ml_dtypes.bfloat16)
    for kl in range(4):
        SEL4[kl, kl * 128:(kl + 1) * 128] = 1.0
    W6 = np.array([[C_PULL, C_PULL, C_PULL, C_PULL, C_PUSH, C_PUSH]],
                  dtype=np.float32)
    return dict(A=A, IOTA=IOTA, E4T=E4T, SEL=SEL, RT=RT,
                ONES_COL=ONES_COL, SEL4=SEL4, W6=W6)


def _kpts_prep(kpts_shard):
    """[B_L, P, 17, 2] -> [4, 400] f32: V[kl, c*200 + (b*5+g)*20 + p] =
    kpts[b, p, min(4g+kl, 16), c]."""
    k_ids = np.minimum(np.arange(P), N_PARTS - 1)  # 0..16,16,16,16
    kp = kpts_shard[:, :, k_ids, :]                # [B_L, P, 20, 2]
    kp = kp.reshape(B_L, P, NG, 4, 2).transpose(3, 4, 0, 2, 1)
    return np.ascontiguousarray(kp.reshape(4, 2 * B_L * NG * P)).astype(
        np.float32
    )


def build_graph():
    nc = Bacc(num_swdge_queues=4, dynamic_dma_scratch_size=65536)

    ebd = nc.declare_dram_parameter("ebd", [B_L, CH, H, W], F32, isOutput=False)
    kp = nc.declare_dram_parameter("kp", [4, 400], F32, isOutput=False)
    A_d = nc.declare_dram_parameter("A", [128, NQ * 160], F32, isOutput=False)
    IO_d = nc.declare_dram_parameter("IOTA", [128, ELEM], F32, isOutput=False)
    E4_d = nc.declare_dram_parameter("E4T", [4, 128], BF16, isOutput=False)
    SEL_d = nc.declare_dram_parameter("SEL", [128, 2 * D], F32, isOutput=False)
    RT_d = nc.declare_dram_parameter("RT", [D, 128], F32, isOutput=False)
    OC_d = nc.declare_dram_parameter("ONES_COL", [128, 1], F32, isOutput=False)
    S4_d = nc.declare_dram_parameter("SEL4", [4, 4 * 128], BF16, isOutput=False)
    W6_d = nc.declare_dram_parameter("W6", [1, 6], F32, isOutput=False)
    out_ext = nc.declare_dram_parameter("out", [1], F32, isOutput=True)

    from contextlib import ExitStack

    ctx = ExitStack()
    with ctx:
        sb = lambda name, shape, dt=F32: ctx.enter_context(
            nc.sbuf_tensor(name, shape, dt)
        )
        ps = lambda name, shape: ctx.enter_context(
            nc.psum_tensor(name, shape, F32)
        )

        Vt = sb("Vt", [4, 400])            # kpts coords (kl x (c,b,g,p))
        U32 = sb("U32", [4, 400], I32)     # scratch int views
        SH = sb("SH", [4, 400], I32)
        YI = sb("YI", [4, 400], I32)       # floor(v*128) as int32
        Yf = sb("Yf", [4, 400])            # floor(v*128) as f32
        Gg = sb("Gg", [4, 200])            # (x >= 64) as 0/1 f32
        Gm = sb("Gm", [4, 200])            # -64*g
        BF1 = sb("BF1", [4, 200])          # 2y
        BASEf = sb("BASEf", [4, 200], BF16)  # 2y + g  (<= 255, bf16-exact)
        XMf = sb("XMf", [4, 200], BF16)    # x % 64   (<= 63, bf16-exact)
        At = sb("At", [128, NQ * 160])     # A const f32
        IDX = sb("IDX", [128, NQ * 160], I16)
        It = sb("It", [128, ELEM])
        E4t = sb("E4t", [4, 128], BF16)
        SELt = sb("SELt", [128, 2 * D])
        RTt = sb("RTt", [D, 128])
        OCt = sb("OCt", [128, 1])
        S4t = sb("S4t", [4, 4 * 128], BF16)
        W6t = sb("W6t", [1, 6])
        G = sb("G", [128, NQ * P * ELEM])  # gathered chunks
        IDXD = sb("IDXD", [128, 8], I16)   # zeroed idx for the warmup gather
        GD = sb("GD", [128, ELEM])         # warmup gather sink
        XbS = sb("XbS", [128, NC])         # xm broadcast to c_local rows
        M1 = sb("M1", [128, (P // 2) * ELEM])  # one-hot scratch (per half)
        P1 = sb("P1", [128, (P // 2) * ELEM])  # product scratch
        E = sb("E", [128, NC])             # extracted vec values
        Mrep = sb("Mrep", [D, NC])         # means replicated over g
        DF = sb("DF", [128, NC])           # E - mean
        T = sb("T", [128, 6])              # cols 0:2 r1, 2:4 r2, 4:6 push
        FW = sb("FW", [1, 6])
        OUTs = sb("OUTs", [1, 1])

        Bps = ps("Bps", [128, 4 * 512])    # base broadcast (bank-padded)
        Xb = ps("Xb", [128, NC])
        Mps = ps("Mps", [D, 2 * P])
        MB = ps("MB", [128, NC])
        F = ps("F", [1, 6])

        sk = ctx.enter_context(nc.semaphore("sk"))   # kpts DMA
        sa = ctx.enter_context(nc.semaphore("sa"))   # A const DMA
        ss = ctx.enter_context(nc.semaphore("ss"))   # small const DMAs
        gds = [[ctx.enter_context(nc.semaphore(f"gd{q}_{h}"))
                for h in range(2)] for q in range(NQ)]
        gw = ctx.enter_context(nc.semaphore("gw"))   # warmup gather DMA
        gwm = ctx.enter_context(nc.semaphore("gwm"))  # warmup memset
        sv = ctx.enter_context(nc.semaphore("sv"))   # vector
        sp = ctx.enter_context(nc.semaphore("sp"))   # PE
        block = ctx.enter_context(nc.Block())

        MS = {}

        @block.vector
        def _(vec):
            AL = mybir.AluOpType
            cnt = [0]

            def fin(inst):
                inst.then_inc(sv, 1)
                cnt[0] += 1

            def w():
                # same-engine RAW/WAR guard: DVE has no pipeline interlocks
                vec.wait_ge(sv, cnt[0])

            fin(nc.vector.memset(T[:], 0.0))
            vec.wait_ge(sk, 16)  # kpts loaded

            # exact floor(v*128) = mant >> min(143 - exp, 31); run the
            # y-half first so BASEf (which gates PE -> idx -> gathers)
            # is ready ~2.5us earlier, then the x-half overlaps PE.
            def floor_half(lo, hi):
                uh = Vt[:, lo:hi].bitcast(I32)
                fin(nc.vector.tensor_scalar(
                    out=U32[:, lo:hi], in0=uh, scalar1=23, scalar2=None,
                    op0=AL.logical_shift_right,
                ))
                w()
                fin(nc.vector.tensor_scalar(
                    out=SH[:, lo:hi], in0=U32[:, lo:hi], scalar1=-1,
                    scalar2=143, op0=AL.mult, op1=AL.add,
                ))
                w()
                fin(nc.vector.tensor_scalar(
                    out=SH[:, lo:hi], in0=SH[:, lo:hi], scalar1=31,
                    scalar2=None, op0=AL.min,
                ))
                w()
                fin(nc.vector.tensor_scalar(
                    out=U32[:, lo:hi], in0=uh, scalar1=0x7FFFFF,
                    scalar2=0x800000, op0=AL.bitwise_and, op1=AL.bitwise_or,
                ))
                w()
                fin(nc.vector.tensor_tensor(
                    out=YI[:, lo:hi], in0=U32[:, lo:hi], in1=SH[:, lo:hi],
                    op=AL.logical_shift_right,
                ))
                w()
                fin(nc.vector.tensor_copy(out=Yf[:, lo:hi],
                                          in_=YI[:, lo:hi]))

            floor_half(0, 200)       # y coords
            fin(nc.vector.tensor_scalar(
                out=Gg[:], in0=Vt[:, 200:400], scalar1=0.5, scalar2=None,
                op0=AL.is_ge,
            ))
            w()
            fin(nc.vector.tensor_scalar(
                out=BF1[:], in0=Yf[:, 0:200], scalar1=2.0, scalar2=None,
                op0=AL.mult,
            ))
            w()
            fin(nc.vector.tensor_tensor(
                out=BASEf[:], in0=BF1[:], in1=Gg[:], op=AL.add
            ))
            MS["base"] = cnt[0]
            floor_half(200, 400)     # x coords (overlaps PE Bps matmuls)
            fin(nc.vector.tensor_scalar(
                out=Gm[:], in0=Gg[:], scalar1=-64.0, scalar2=None, op0=AL.mult
            ))
            w()
            fin(nc.vector.tensor_tensor(
                out=XMf[:], in0=Yf[:, 200:400], in1=Gm[:], op=AL.add
            ))
            MS["xm"] = cnt[0]
            # idx = A + Bps  (even/odd cols; in1 reads PSUM with 3D AP),
            # emitted in two q-halves so the first gathers can issue early
            vec.wait_ge(sa, 16)   # At
            vec.wait_ge(sp, 4)    # Bps
            bp0 = Bps[:]
            HQ = NQ // 2
            for qh in range(2):
                for par in range(2):
                    # out/in0 enumerate (q, p, kl) at parity `par`;
                    # in1 reads Bps[c, kl*512 + q*20 + p] in the same order.
                    in1 = AP(bp0.tensor, bp0.offset + qh * HQ * P,
                             [bp0.ap[0], [20, HQ], [1, P], [512, 4]])
                    out = AP(IDX[:].tensor,
                             IDX[:].offset + par + qh * HQ * 160,
                             [IDX[:].ap[0], [160, HQ], [8, P], [2, 4]])
                    in0 = AP(At[:].tensor,
                             At[:].offset + par + qh * HQ * 160,
                             [At[:].ap[0], [160, HQ], [8, P], [2, 4]])
                    fin(nc.vector.tensor_tensor(
                        out=out, in0=in0, in1=in1, op=AL.add
                    ))
                if qh == 0:
                    MS["idx0"] = cnt[0]
            MS["idxdone"] = cnt[0]
            vec.wait_ge(sp, 5)
            fin(nc.vector.tensor_copy(out=XbS[:], in_=Xb[:]))
            MS["eslice"] = []
            # per-half extraction: one-hot compare + multiply + reduce
            HP = P // 2
            for q in range(NQ):
                for h in range(2):
                    vec.wait_ge(gds[q][h], 16)
                    if q == 0 and h == 0:
                        vec.wait_ge(ss, 112)  # IOTA et al
                    it = It[:]
                    iota_b = AP(it.tensor, it.offset,
                                [it.ap[0], [0, HP], [1, ELEM]])
                    xb = XbS[:, q * P + h * HP:q * P + (h + 1) * HP]
                    xb_b = AP(xb.tensor, xb.offset,
                              [xb.ap[0], [1, HP], [0, ELEM]])
                    w()
                    fin(nc.vector.tensor_tensor(
                        out=M1[:].rearrange("p (a b) -> p a b", b=ELEM),
                        in0=iota_b, in1=xb_b, op=AL.is_equal,
                    ))
                    w()
                    goff = (q * P + h * HP) * ELEM
                    fin(nc.vector.tensor_tensor(
                        out=P1[:], in0=G[:, goff:goff + HP * ELEM],
                        in1=M1[:], op=AL.mult,
                    ))
                    w()
                    fin(nc.vector.tensor_reduce(
                        out=E[:, q * P + h * HP:q * P + (h + 1) * HP],
                        in_=P1[:].rearrange("p (a b) -> p a b", b=ELEM),
                        axis=mybir.AxisListType.X, op=AL.add,
                    ))
                MS["eslice"].append(cnt[0])
            # tail: means -> pull/push partial sums into T's columns
            vec.wait_ge(sp, 15)
            mp = Mps[:]
            fin(nc.vector.tensor_copy(
                out=Mrep[:],
                in_=AP(mp.tensor, mp.offset,
                       [mp.ap[0], [P, 2], [0, NG], [1, P]]),
            ))
            MS["mrep"] = cnt[0]
            vec.wait_ge(sp, 16)
            fin(nc.vector.tensor_tensor(
                out=DF[:], in0=E[:], in1=MB[:], op=AL.subtract
            ))
            df = DF[:]
            w()
            fin(nc.vector.tensor_reduce(
                out=T[:, 0:2],
                in_=AP(df.tensor, df.offset,
                       [df.ap[0], [5 * P, 2], [P, 4], [1, P]]),
                axis=mybir.AxisListType.XY, op=AL.add,
                apply_absolute_value=True,
            ))
            df32 = DF[0:32, :]
            w()
            fin(nc.vector.tensor_reduce(
                out=T[0:32, 2:4],
                in_=AP(df32.tensor, df32.offset + 4 * P,
                       [df32.ap[0], [5 * P, 2], [1, P]]),
                axis=mybir.AxisListType.X, op=AL.add,
                apply_absolute_value=True,
            ))
            # push: pairwise |m_p - m_q| — in0 from Mps (PSUM), in1 from the
            # SBUF replica Mrep (g=0 block holds means at cols b*100+p)
            mr = Mrep[:]
            in0 = AP(mp.tensor, mp.offset, [mp.ap[0], [P, 2], [1, P], [0, P]])
            in1 = AP(mr.tensor, mr.offset,
                     [mr.ap[0], [5 * P, 2], [0, P], [1, P]])
            # G is fully consumed by now — reuse it as pairwise-diff scratch
            pd_out = G[0:32, 0:2 * P * P].rearrange(
                "p (a b c) -> p a b c", a=2, b=P
            )
            fin(nc.vector.tensor_tensor(
                out=pd_out, in0=in0, in1=in1, op=AL.subtract
            ))
            w()
            fin(nc.vector.tensor_reduce(
                out=T[0:32, 4:6], in_=pd_out, axis=mybir.AxisListType.XY,
                op=AL.add, apply_absolute_value=True,
            ))
            MS["tdone"] = cnt[0]
            vec.wait_ge(sp, 17)
            fin(nc.vector.tensor_tensor(
                out=FW[:], in0=F[:], in1=W6t[:], op=AL.mult
            ))
            w()
            fin(nc.vector.tensor_reduce(
                out=OUTs[:], in_=FW[:], axis=mybir.AxisListType.X, op=AL.add
            ))
            MS["loss"] = cnt[0]

        @block.tensor
        def _(pe):
            # Bps: broadcast base rows to 128 partitions (4 bf16 matmuls)
            pe.wait_ge(sv, MS["base"])
            pe.wait_ge(ss, 112)
            for kl in range(4):
                nc.tensor.matmul(
                    out=Bps[:, kl * 512:kl * 512 + 200],
                    lhsT=S4t[:, kl * 128:(kl + 1) * 128],
                    rhs=BASEf[:],
                    start=True, stop=True,
                ).then_inc(sp, 1)
            # Xb: broadcast xm to c_local partitions
            pe.wait_ge(sv, MS["xm"])
            nc.tensor.matmul(
                out=Xb[:], lhsT=E4t[:], rhs=XMf[:], start=True, stop=True
            ).then_inc(sp, 1)
            # means: accumulate per-b over groups
            for q in range(NQ):
                b, g = divmod(q, NG)
                pe.wait_ge(sv, MS["eslice"][q])
                nc.tensor.matmul(
                    out=Mps[:, b * P:(b + 1) * P],
                    lhsT=SELt[:, D:2 * D] if g == NG - 1 else SELt[:, 0:D],
                    rhs=E[:, q * P:(q + 1) * P],
                    start=(g == 0), stop=(g == NG - 1),
                ).then_inc(sp, 1)
            # MB: broadcast means to [128, NC]
            pe.wait_ge(sv, MS["mrep"])
            nc.tensor.matmul(
                out=MB[:], lhsT=RTt[:], rhs=Mrep[:], start=True, stop=True
            ).then_inc(sp, 1)
            # F: partition sum of T
            pe.wait_ge(sv, MS["tdone"])
            nc.tensor.matmul(
                out=F[:], lhsT=OCt[:], rhs=T[:], start=True, stop=True
            ).then_inc(sp, 1)

        @block.gpsimd
        def _(gpsimd):
            # Warmup: a tiny gather issued first so Bacc places the mlp
            # ucode library load here, overlapping the index front-end
            # instead of serializing after it.
            gpsimd.memset(IDXD[:], 0).then_inc(gwm, 1)
            gpsimd.wait_ge(gwm, 1)
            gpsimd.dma_gather(
                out_ap=GD[:].rearrange("p (a b) -> p a b", b=ELEM),
                in_ap=AP(ebd, 0, [[ELEM, 128], [1, ELEM]]),
                idxs_ap=IDXD[:],
                num_idxs=128,
                num_idxs_reg=128,
                elem_size=ELEM,
                single_packet=False,
                queue_num=0,
            ).then_inc(gw, 16)
            gpsimd.wait_ge(sv, MS["idx0"])
            qq = 0
            for q in range(NQ):
                if q == NQ // 2:
                    gpsimd.wait_ge(sv, MS["idxdone"])
                b, g = divmod(q, NG)
                base = b * CH * PLANE + g * 128 * PLANE
                nrows = (CHUNKS_PER_PLANE * 128) if g < NG - 1 else (
                    CHUNKS_PER_PLANE * 32
                )
                in_ap = AP(ebd, base, [[ELEM, nrows], [1, ELEM]])
                # two halves (10 people each) spread over the 4 SWDGE queues
                for h in range(2):
                    half = NI // 2
                    off = q * P * ELEM + h * (P // 2) * ELEM
                    out_ap = G[:, off:off + (P // 2) * ELEM].rearrange(
                        "p (a b) -> p a b", b=ELEM
                    )
                    gpsimd.dma_gather(
                        out_ap=out_ap,
                        in_ap=in_ap,
                        idxs_ap=IDX[:, q * 160 + h * 80:
                                    q * 160 + (h + 1) * 80],
                        num_idxs=half,
                        num_idxs_reg=half,
                        elem_size=ELEM,
                        single_packet=False,
                        queue_num=qq % 4,
                    ).then_inc(gds[q][h], 16)
                    qq += 1

        @block.sync
        def _(sync):
            sync.dma_start(out=Vt[:], in_=kp[:]).then_inc(sk, 16)
            sync.dma_start(out=At[:], in_=A_d[:]).then_inc(sa, 16)
            sync.wait_ge(sv, MS["loss"])
            sync.dma_start(out=out_ext[:], in_=OUTs[0:1, 0:1]).then_inc(sk, 16)

        @block.scalar
        def _(scalar):
            for dst, src in (
                (It, IO_d), (E4t, E4_d), (SELt, SEL_d), (RTt, RT_d),
                (OCt, OC_d), (S4t, S4_d), (W6t, W6_d),
            ):
                scalar.dma_start(out=dst[:], in_=src[:]).then_inc(ss, 16)

    return nc


_CONSTS = None


def _run(ebd_batch: np.ndarray, kpts: np.ndarray, trace: bool = False):
    from concourse.bass_utils import run_bass_kernel_spmd

    global _CONSTS
    if _CONSTS is None:
        _CONSTS = _host_consts()
    consts = _CONSTS

    nc = build_graph()
    nc.finalize()

    in_maps = []
    for c in range(N_CORES):
        sl = slice(c * B_L, (c + 1) * B_L)
        m = dict(
            ebd=np.ascontiguousarray(ebd_batch[sl]).astype(np.float32),
            kp=_kpts_prep(kpts[sl].astype(np.float32)),
            A=consts["A"], IOTA=consts["IOTA"], E4T=consts["E4T"],
            SEL=consts["SEL"], RT=consts["RT"],
            ONES_COL=consts["ONES_COL"], SEL4=consts["SEL4"],
            W6=consts["W6"],
        )
        in_maps.append(m)

    res = run_bass_kernel_spmd(
        nc, in_maps, core_ids=list(range(N_CORES)), trace=trace
    )
    total = sum(float(res.results[c]["out"][0]) for c in range(N_CORES))
    return np.float32(total / B), res


def kernel(ebd_batch: np.ndarray, kpts: np.ndarray) -> np.ndarray:
    return _run(ebd_batch, kpts, trace=False)[0]


if __name__ == "__main__":
    np.random.seed(0)
    ebd = np.random.randn(B, CH, H, W).astype(np.float32)
    kk = np.random.rand(B, P, N_PARTS, 2).astype(np.float32)
    print(kernel(ebd, kk))


# revision 34
# speedup vs baseline: 1.2153x; 1.0080x over previous
"""AELoss (associative embedding loss) distributed Bass kernel for TRN2.

Problem: ebd_batch [16, 544, 128, 128] f32, kpts [16, 20, 17, 2] f32.
  vecs[b,p,k,:] = ebd[b, k*32:(k+1)*32, y(b,p,k), x(b,p,k)]  (y=floor(ky*128))
  means = vecs.mean(parts); pull/push L1 stats -> scalar loss.

Strategy: pure data parallel over batch (2 batches/core on 8 cores). The
essential data is only 5440 32-float vectors out of 570MB, so instead of
streaming, each core dma_gathers 256B chunks (the minimum indexable unit)
around each needed element — 20 calls spread over the 4 SWDGE queues (each
queue drains at roughly one SDMA engine's line rate) — then extracts the
exact element on-chip with a one-hot compare+reduce overlapped per call.
Indices are computed on device from kpts (exact floor via int bit
manipulation). Per-core partial losses are summed on host.

Layout cheat sheet (per core):
  B_L=2 local batches, P=20 people, parts padded 17->20 = 5 groups (g) of 4
  (kl), D=32. call q = b*5+g (2 gather halves each). c_local = kl*32+d.
  E[c_local, qp] with qp = q*20+p holds vec(b, p, k=4g+kl, d).
  gather idx (int16) = c_eff*256 + 2*y + (x>=64), window = 128 planes (8MB).
"""

import sys

sys.path.insert(0, "/opt/trn_rl_repo")

import numpy as np
import ml_dtypes

import concourse.mybir as mybir
from concourse.ap import AP
from concourse.bacc import Bacc

F32 = mybir.dt.float32
BF16 = mybir.dt.bfloat16
I32 = mybir.dt.int32
I16 = mybir.dt.int16

B, CH, H, W = 16, 544, 128, 128
D = 32
N_PARTS = 17
P = 20
N_CORES = 8
B_L = B // N_CORES          # 2 local batches
NG = 5                      # part groups of 4 (parts padded to 20)
NQ = B_L * NG               # 10 logical gather calls (x2 halves)
NI = P * 128                # 2560 idxs per logical call
NC = NQ * P                 # 200 E columns
PLANE = H * W
ELEM = 64                   # gathered chunk = 64 f32 = 256B
CHUNKS_PER_PLANE = PLANE // ELEM  # 256

# loss = mean_b (push_b + pull_b)/2 ; per-core out = sum_local_b (...)/2,
# host divides by B. pull_b = sum_all|d|/544 ; push_b = sum|md|/12800.
C_PULL = 1.0 / (544.0 * 2.0)
C_PUSH = 1.0 / (12800.0 * 2.0)


def _host_consts():
    """Constant tensors DMA'd to every core."""
    # A term of the gather index: A[Pr, J] = c_eff * 256 where
    # i = (J%160)*16 + Pr%16, c_local = i % 128, call q = J // 160,
    # c_eff = c_local%32 if q%5==4 else c_local  (last group: only part 16
    # is real; pad kls duplicate part16's chunk so addresses stay in range).
    Pr = np.arange(128)[:, None]
    J = np.arange(NQ * 160)[None, :]
    i = (J % 160) * 16 + (Pr % 16)
    c_local = i % 128
    q = J // 160
    c_eff = np.where(q % NG == NG - 1, c_local % 32, c_local)
    A = (c_eff * 256).astype(np.float32)

    IOTA = np.tile(np.arange(ELEM, dtype=np.float32)[None, :], (128, 1))

    # E4T[kl, c] = 1 if c//32 == kl   (broadcasts xm to 128 partitions)
    E4T = (np.arange(128)[None, :] // 32 == np.arange(4)[:, None]).astype(
        ml_dtypes.bfloat16
    )

    # SEL [128, 64]: cols 0:32 full-group mean weights, cols 32:64 last group
    c = np.arange(128)[:, None]
    d = np.arange(D)[None, :]
    sel_full = (c % 32 == d).astype(np.float32) / N_PARTS
    sel_last = sel_full * (c < 32)
    SEL = np.concatenate([sel_full, sel_last], axis=1).astype(np.float32)

    # RT[d, c] = 1 if c%32 == d  (broadcast means over part groups)
    RT = (np.arange(128)[None, :] % 32 == np.arange(D)[:, None]).astype(
        np.float32
    )

    ONES_COL = np.ones((128, 1), dtype=np.float32)   # lhsT for partition sum
    # SEL4[j, kl*128 + c] = 1 if j == kl : row-broadcast selector
    SEL4 = np.zeros((4, 4 * 128), dtype=ml_dtypes.bfloat16)
    for kl in range(4):
        SEL4[kl, kl * 128:(kl + 1) * 128] = 1.0
    W6 = np.array([[C_PULL, C_PULL, C_PULL, C_PULL, C_PUSH, C_PUSH]],
                  dtype=np.float32)
    return dict(A=A, IOTA=IOTA, E4T=E4T, SEL=SEL, RT=RT,
                ONES_COL=ONES_COL, SEL4=SEL4, W6=W6)


def _kpts_prep(kpts_shard):
    """[B_L, P, 17, 2] -> [4, 400] f32: V[kl, c*200 + (b*5+g)*20 + p] =
    kpts[b, p, min(4g+kl, 16), c]."""
    k_ids = np.minimum(np.arange(P), N_PARTS - 1)  # 0..16,16,16,16
    kp = kpts_shard[:, :, k_ids, :]                # [B_L, P, 20, 2]
    kp = kp.reshape(B_L, P, NG, 4, 2).transpose(3, 4, 0, 2, 1)
    return np.ascontiguousarray(kp.reshape(4, 2 * B_L * NG * P)).astype(
        np.float32
    )


def build_graph():
    nc = Bacc(num_swdge_queues=4, dynamic_dma_scratch_size=65536)

    ebd = nc.declare_dram_parameter("ebd", [B_L, CH, H, W], F32, isOutput=False)
    kp = nc.declare_dram_parameter("kp", [4, 400], F32, isOutput=False)
    A_d = nc.declare_dram_parameter("A", [128, NQ * 160], F32, isOutput=False)
    IO_d = nc.declare_dram_parameter("IOTA", [128, ELEM], F32, isOutput=False)
    E4_d = nc.declare_dram_parameter("E4T", [4, 128], BF16, isOutput=False)
    SEL_d = nc.declare_dram_parameter("SEL", [128, 2 * D], F32, isOutput=False)
    RT_d = nc.declare_dram_parameter("RT", [D, 128], F32, isOutput=False)
    OC_d = nc.declare_dram_parameter("ONES_COL", [128, 1], F32, isOutput=False)
    S4_d = nc.declare_dram_parameter("SEL4", [4, 4 * 128], BF16, isOutput=False)
    W6_d = nc.declare_dram_parameter("W6", [1, 6], F32, isOutput=False)
    out_ext = nc.declare_dram_parameter("out", [1], F32, isOutput=True)

    from contextlib import ExitStack

    ctx = ExitStack()
    with ctx:
        sb = lambda name, shape, dt=F32: ctx.enter_context(
            nc.sbuf_tensor(name, shape, dt)
        )
        ps = lambda name, shape: ctx.enter_context(
            nc.psum_tensor(name, shape, F32)
        )

        Vt = sb("Vt", [4, 400])            # kpts coords (kl x (c,b,g,p))
        U32 = sb("U32", [4, 400], I32)     # scratch int views
        SH = sb("SH", [4, 400], I32)
        YI = sb("YI", [4, 400], I32)       # floor(v*128) as int32
        Yf = sb("Yf", [4, 400])            # floor(v*128) as f32
        Gg = sb("Gg", [4, 200])            # (x >= 64) as 0/1 f32
        Gm = sb("Gm", [4, 200])            # -64*g
        BF1 = sb("BF1", [4, 200])          # 2y
        BASEf = sb("BASEf", [4, 200], BF16)  # 2y + g  (<= 255, bf16-exact)
        XMf = sb("XMf", [4, 200], BF16)    # x % 64   (<= 63, bf16-exact)
        At = sb("At", [128, NQ * 160])     # A const f32
        IDX = sb("IDX", [128, NQ * 160], I16)
        It = sb("It", [128, ELEM])
        E4t = sb("E4t", [4, 128], BF16)
        SELt = sb("SELt", [128, 2 * D])
        RTt = sb("RTt", [D, 128])
        OCt = sb("OCt", [128, 1])
        S4t = sb("S4t", [4, 4 * 128], BF16)
        W6t = sb("W6t", [1, 6])
        G = sb("G", [128, NQ * P * ELEM])  # gathered chunks
        IDXD = sb("IDXD", [128, 8], I16)   # zeroed idx for the warmup gather
        GD = sb("GD", [128, ELEM])         # warmup gather sink
        XbS = sb("XbS", [128, NC])         # xm broadcast to c_local rows
        M1 = sb("M1", [128, (P // 2) * ELEM])  # one-hot scratch (per half)
        P1 = sb("P1", [128, (P // 2) * ELEM])  # product scratch
        E = sb("E", [128, NC])             # extracted vec values
        Mrep = sb("Mrep", [D, NC])         # means replicated over g
        DF = sb("DF", [128, NC])           # E - mean
        T = sb("T", [128, 6])              # cols 0:2 r1, 2:4 r2, 4:6 push
        FW = sb("FW", [1, 6])
        OUTs = sb("OUTs", [1, 1])

        Bps = ps("Bps", [128, 4 * 512])    # base broadcast (bank-padded)
        Xb = ps("Xb", [128, NC])
        Mps = ps("Mps", [D, 2 * P])
        MB = ps("MB", [128, NC])
        F = ps("F", [1, 6])

        sk = ctx.enter_context(nc.semaphore("sk"))   # kpts DMA
        sa = ctx.enter_context(nc.semaphore("sa"))   # A const DMA
        ss = ctx.enter_context(nc.semaphore("ss"))   # small const DMAs
        gds = [[ctx.enter_context(nc.semaphore(f"gd{q}_{h}"))
                for h in range(2)] for q in range(NQ)]
        gw = ctx.enter_context(nc.semaphore("gw"))   # warmup gather DMA
        gwm = ctx.enter_context(nc.semaphore("gwm"))  # warmup memset
        sv = ctx.enter_context(nc.semaphore("sv"))   # vector
        sp = ctx.enter_context(nc.semaphore("sp"))   # PE
        block = ctx.enter_context(nc.Block())

        MS = {}

        @block.vector
        def _(vec):
            AL = mybir.AluOpType
            cnt = [0]

            def fin(inst):
                inst.then_inc(sv, 1)
                cnt[0] += 1

            def w():
                # same-engine RAW/WAR guard: DVE has no pipeline interlocks
                vec.wait_ge(sv, cnt[0])

            fin(nc.vector.memset(T[:], 0.0))
            vec.wait_ge(sk, 16)  # kpts loaded

            # exact floor(v*128) = mant >> min(143 - exp, 31); run the
            # y-half first so BASEf (which gates PE -> idx -> gathers)
            # is ready ~2.5us earlier, then the x-half overlaps PE.
            def floor_half(lo, hi):
                uh = Vt[:, lo:hi].bitcast(I32)
                fin(nc.vector.tensor_scalar(
                    out=U32[:, lo:hi], in0=uh, scalar1=23, scalar2=None,
                    op0=AL.logical_shift_right,
                ))
                w()
                fin(nc.vector.tensor_scalar(
                    out=SH[:, lo:hi], in0=U32[:, lo:hi], scalar1=-1,
                    scalar2=143, op0=AL.mult, op1=AL.add,
                ))
                w()
                fin(nc.vector.tensor_scalar(
                    out=SH[:, lo:hi], in0=SH[:, lo:hi], scalar1=31,
                    scalar2=None, op0=AL.min,
                ))
                w()
                fin(nc.vector.tensor_scalar(
                    out=U32[:, lo:hi], in0=uh, scalar1=0x7FFFFF,
                    scalar2=0x800000, op0=AL.bitwise_and, op1=AL.bitwise_or,
                ))
                w()
                fin(nc.vector.tensor_tensor(
                    out=YI[:, lo:hi], in0=U32[:, lo:hi], in1=SH[:, lo:hi],
                    op=AL.logical_shift_right,
                ))
                w()
                fin(nc.vector.tensor_copy(out=Yf[:, lo:hi],
                                          in_=YI[:, lo:hi]))

            floor_half(0, 200)       # y coords
            fin(nc.vector.tensor_scalar(
                out=Gg[:], in0=Vt[:, 200:400], scalar1=0.5, scalar2=None,
                op0=AL.is_ge,
            ))
            w()
            fin(nc.vector.tensor_scalar(
                out=BF1[:], in0=Yf[:, 0:200], scalar1=2.0, scalar2=None,
                op0=AL.mult,
            ))
            w()
            fin(nc.vector.tensor_tensor(
                out=BASEf[:], in0=BF1[:], in1=Gg[:], op=AL.add
            ))
            MS["base"] = cnt[0]
            floor_half(200, 400)     # x coords (overlaps PE Bps matmuls)
            fin(nc.vector.tensor_scalar(
                out=Gm[:], in0=Gg[:], scalar1=-64.0, scalar2=None, op0=AL.mult
            ))
            w()
            fin(nc.vector.tensor_tensor(
                out=XMf[:], in0=Yf[:, 200:400], in1=Gm[:], op=AL.add
            ))
            MS["xm"] = cnt[0]
            # idx = A + Bps  (even/odd cols; in1 reads PSUM with 3D AP),
            # emitted in two q-halves so the first gathers can issue early
            vec.wait_ge(sa, 16)   # At
            vec.wait_ge(sp, 4)    # Bps
            bp0 = Bps[:]
            HQ = NQ // 2
            for qh in range(2):
                for par in range(2):
                    # out/in0 enumerate (q, p, kl) at parity `par`;
                    # in1 reads Bps[c, kl*512 + q*20 + p] in the same order.
                    in1 = AP(bp0.tensor, bp0.offset + qh * HQ * P,
                             [bp0.ap[0], [20, HQ], [1, P], [512, 4]])
                    out = AP(IDX[:].tensor,
                             IDX[:].offset + par + qh * HQ * 160,
                             [IDX[:].ap[0], [160, HQ], [8, P], [2, 4]])
                    in0 = AP(At[:].tensor,
                             At[:].offset + par + qh * HQ * 160,
                             [At[:].ap[0], [160, HQ], [8, P], [2, 4]])
                    fin(nc.vector.tensor_tensor(
                        out=out, in0=in0, in1=in1, op=AL.add
                    ))
                if qh == 0:
                    MS["idx0"] = cnt[0]
            MS["idxdone"] = cnt[0]
            vec.wait_ge(sp, 5)
            fin(nc.vector.tensor_copy(out=XbS[:], in_=Xb[:]))
            MS["eslice"] = []
            # per-half extraction: one-hot compare + multiply + reduce
            HP = P // 2
            for q in range(NQ):
                for h in range(2):
                    vec.wait_ge(gds[q][h], 16)
                    if q == 0 and h == 0:
                        vec.wait_ge(ss, 112)  # IOTA et al
                    it = It[:]
                    iota_b = AP(it.tensor, it.offset,
                                [it.ap[0], [0, HP], [1, ELEM]])
                    xb = XbS[:, q * P + h * HP:q * P + (h + 1) * HP]
                    xb_b = AP(xb.tensor, xb.offset,
                              [xb.ap[0], [1, HP], [0, ELEM]])
                    w()
                    fin(nc.vector.tensor_tensor(
                        out=M1[:].rearrange("p (a b) -> p a b", b=ELEM),
                        in0=iota_b, in1=xb_b, op=AL.is_equal,
                    ))
                    w()
                    goff = (q * P + h * HP) * ELEM
                    fin(nc.vector.tensor_tensor(
                        out=P1[:], in0=G[:, goff:goff + HP * ELEM],
                        in1=M1[:], op=AL.mult,
                    ))
                    w()
                    fin(nc.vector.tensor_reduce(
                        out=E[:, q * P + h * HP:q * P + (h + 1) * HP],
                        in_=P1[:].rearrange("p (a b) -> p a b", b=ELEM),
                        axis=mybir.AxisListType.X, op=AL.add,
                    ))
                MS["eslice"].append(cnt[0])
            # tail: means -> pull/push partial sums into T's columns
            vec.wait_ge(sp, 15)
            mp = Mps[:]
            fin(nc.vector.tensor_copy(
                out=Mrep[:],
                in_=AP(mp.tensor, mp.offset,
                       [mp.ap[0], [P, 2], [0, NG], [1, P]]),
            ))
            MS["mrep"] = cnt[0]
            # push first: pairwise |m_p - m_q| needs only the means (sp 15),
            # so its ~2us of DVE work hides the MB broadcast matmul latency.
            # in0 from Mps (PSUM), in1 from the SBUF replica Mrep (g=0 block
            # holds means at cols b*100+p).
            mr = Mrep[:]
            in0 = AP(mp.tensor, mp.offset, [mp.ap[0], [P, 2], [1, P], [0, P]])
            in1 = AP(mr.tensor, mr.offset,
                     [mr.ap[0], [5 * P, 2], [0, P], [1, P]])
            # G is fully consumed by now — reuse it as pairwise-diff scratch
            pd_out = G[0:32, 0:2 * P * P].rearrange(
                "p (a b c) -> p a b c", a=2, b=P
            )
            w()
            fin(nc.vector.tensor_tensor(
                out=pd_out, in0=in0, in1=in1, op=AL.subtract
            ))
            w()
            fin(nc.vector.tensor_reduce(
                out=T[0:32, 4:6], in_=pd_out, axis=mybir.AxisListType.XY,
                op=AL.add, apply_absolute_value=True,
            ))
            vec.wait_ge(sp, 16)
            fin(nc.vector.tensor_tensor(
                out=DF[:], in0=E[:], in1=MB[:], op=AL.subtract
            ))
            df = DF[:]
            w()
            fin(nc.vector.tensor_reduce(
                out=T[:, 0:2],
                in_=AP(df.tensor, df.offset,
                       [df.ap[0], [5 * P, 2], [P, 4], [1, P]]),
                axis=mybir.AxisListType.XY, op=AL.add,
                apply_absolute_value=True,
            ))
            df32 = DF[0:32, :]
            w()
            fin(nc.vector.tensor_reduce(
                out=T[0:32, 2:4],
                in_=AP(df32.tensor, df32.offset + 4 * P,
                       [df32.ap[0], [5 * P, 2], [1, P]]),
                axis=mybir.AxisListType.X, op=AL.add,
                apply_absolute_value=True,
            ))
            MS["tdone"] = cnt[0]
            vec.wait_ge(sp, 17)
            fin(nc.vector.tensor_tensor(
                out=FW[:], in0=F[:], in1=W6t[:], op=AL.mult
            ))
            w()
            fin(nc.vector.tensor_reduce(
                out=OUTs[:], in_=FW[:], axis=mybir.AxisListType.X, op=AL.add
            ))
            MS["loss"] = cnt[0]

        @block.tensor
        def _(pe):
            # Bps: broadcast base rows to 128 partitions (4 bf16 matmuls)
            pe.wait_ge(sv, MS["base"])
            pe.wait_ge(ss, 112)
            for kl in range(4):
                nc.tensor.matmul(
                    out=Bps[:, kl * 512:kl * 512 + 200],
                    lhsT=S4t[:, kl * 128:(kl + 1) * 128],
                    rhs=BASEf[:],
                    start=True, stop=True,
                ).then_inc(sp, 1)
            # Xb: broadcast xm to c_local partitions
            pe.wait_ge(sv, MS["xm"])
            nc.tensor.matmul(
                out=Xb[:], lhsT=E4t[:], rhs=XMf[:], start=True, stop=True
            ).then_inc(sp, 1)
            # means: accumulate per-b over groups
            for q in range(NQ):
                b, g = divmod(q, NG)
                pe.wait_ge(sv, MS["eslice"][q])
                nc.tensor.matmul(
                    out=Mps[:, b * P:(b + 1) * P],
                    lhsT=SELt[:, D:2 * D] if g == NG - 1 else SELt[:, 0:D],
                    rhs=E[:, q * P:(q + 1) * P],
                    start=(g == 0), stop=(g == NG - 1),
                ).then_inc(sp, 1)
            # MB: broadcast means to [128, NC]
            pe.wait_ge(sv, MS["mrep"])
            nc.tensor.matmul(
                out=MB[:], lhsT=RTt[:], rhs=Mrep[:], start=True, stop=True
            ).then_inc(sp, 1)
            # F: partition sum of T
            pe.wait_ge(sv, MS["tdone"])
            nc.tensor.matmul(
                out=F[:], lhsT=OCt[:], rhs=T[:], start=True, stop=True
            ).then_inc(sp, 1)

        @block.gpsimd
        def _(gpsimd):
            # Warmup: a tiny gather issued first so Bacc places the mlp
            # ucode library load here, overlapping the index front-end
            # instead of serializing after it.
            gpsimd.memset(IDXD[:], 0).then_inc(gwm, 1)
            gpsimd.wait_ge(gwm, 1)
            gpsimd.dma_gather(
                out_ap=GD[:].rearrange("p (a b) -> p a b", b=ELEM),
                in_ap=AP(ebd, 0, [[ELEM, 128], [1, ELEM]]),
                idxs_ap=IDXD[:],
                num_idxs=128,
                num_idxs_reg=128,
                elem_size=ELEM,
                single_packet=False,
                queue_num=0,
            ).then_inc(gw, 16)
            gpsimd.wait_ge(sv, MS["idx0"])
            qq = 0
            for q in range(NQ):
                if q == NQ // 2:
                    gpsimd.wait_ge(sv, MS["idxdone"])
                b, g = divmod(q, NG)
                base = b * CH * PLANE + g * 128 * PLANE
                nrows = (CHUNKS_PER_PLANE * 128) if g < NG - 1 else (
                    CHUNKS_PER_PLANE * 32
                )
                in_ap = AP(ebd, base, [[ELEM, nrows], [1, ELEM]])
                # two halves (10 people each) spread over the 4 SWDGE queues
                for h in range(2):
                    half = NI // 2
                    off = q * P * ELEM + h * (P // 2) * ELEM
                    out_ap = G[:, off:off + (P // 2) * ELEM].rearrange(
                        "p (a b) -> p a b", b=ELEM
                    )
                    gpsimd.dma_gather(
                        out_ap=out_ap,
                        in_ap=in_ap,
                        idxs_ap=IDX[:, q * 160 + h * 80:
                                    q * 160 + (h + 1) * 80],
                        num_idxs=half,
                        num_idxs_reg=half,
                        elem_size=ELEM,
                        single_packet=False,
                        queue_num=qq % 4,
                    ).then_inc(gds[q][h], 16)
                    qq += 1

        @block.sync
        def _(sync):
            sync.dma_start(out=Vt[:], in_=kp[:]).then_inc(sk, 16)
            sync.dma_start(out=At[:], in_=A_d[:]).then_inc(sa, 16)
            sync.wait_ge(sv, MS["loss"])
            sync.dma_start(out=out_ext[:], in_=OUTs[0:1, 0:1]).then_inc(sk, 16)

        @block.scalar
        def _(scalar):
            for dst, src in (
                (It, IO_d), (E4t, E4_d), (SELt, SEL_d), (RTt, RT_d),
                (OCt, OC_d), (S4t, S4_d), (W6t, W6_d),
            ):
                scalar.dma_start(out=dst[:], in_=src[:]).then_inc(ss, 16)

    return nc


_CONSTS = None


def _run(ebd_batch: np.ndarray, kpts: np.ndarray, trace: bool = False):
    from concourse.bass_utils import run_bass_kernel_spmd

    global _CONSTS
    if _CONSTS is None:
        _CONSTS = _host_consts()
    consts = _CONSTS

    nc = build_graph()
    nc.finalize()

    in_maps = []
    for c in range(N_CORES):
        sl = slice(c * B_L, (c + 1) * B_L)
        m = dict(
            ebd=np.ascontiguousarray(ebd_batch[sl]).astype(np.float32),
            kp=_kpts_prep(kpts[sl].astype(np.float32)),
            A=consts["A"], IOTA=consts["IOTA"], E4T=consts["E4T"],
            SEL=consts["SEL"], RT=consts["RT"],
            ONES_COL=consts["ONES_COL"], SEL4=consts["SEL4"],
            W6=consts["W6"],
        )
        in_maps.append(m)

    res = run_bass_kernel_spmd(
        nc, in_maps, core_ids=list(range(N_CORES)), trace=trace
    )
    total = sum(float(res.results[c]["out"][0]) for c in range(N_CORES))
    return np.float32(total / B), res


def kernel(ebd_batch: np.ndarray, kpts: np.ndarray) -> np.ndarray:
    return _run(ebd_batch, kpts, trace=False)[0]


if __name__ == "__main__":
    np.random.seed(0)
    ebd = np.random.randn(B, CH, H, W).astype(np.float32)
    kk = np.random.rand(B, P, N_PARTS, 2).astype(np.float32)
    print(kernel(ebd, kk))
